# revision 1
# baseline (speedup 1.0000x reference)
"""Trainium2 Bass kernel for nn_Equalization (per-image per-channel histogram
equalization of [64, 512, 512, 3] int32 images, values 0..255).

Sharding: pure data parallelism over the batch dimension — 8 images per
NeuronCore, 8 cores. Each core computes, for each of its 8 images and each
RGB channel: a 256-bin histogram (nibble-split one-hots contracted on the
tensor engine into a (hi, lo) joint-count PSUM tile), the exact
cumsum-based equalization LUT (integer-exact fp32 arithmetic with
floor-division fixups), and the LUT application (one-hot select against the
broadcast LUT), all on-device.

self-contained: builds the Bass module, shards inputs, runs via
run_bass_kernel_spmd on cores 0-7, gathers the full output.
"""
import numpy as np

from concourse import bacc, mybir
import concourse.tile as tile
from concourse.bass_utils import run_bass_kernel_spmd

F32 = mybir.dt.float32
BF16 = mybir.dt.bfloat16
U8 = mybir.dt.uint8
I32 = mybir.dt.int32
AL = mybir.AluOpType
AX = mybir.AxisListType

P = 128
NPIX = 512 * 512          # pixels per channel
FW = NPIX * 3 // P        # 6144 int32 per partition per image
CW = NPIX // P            # 2048 pixels per partition per channel
N_CORES = 8
N_IMG = 8                 # images per core


def _build_kernel(n_img=N_IMG, j_hist=128, repeat=1):
    R = 3 * n_img  # number of (image, channel) units
    assert R <= 64
    nc = bacc.Bacc("TRN2", target_bir_lowering=False, debug=False,
                   num_devices=N_CORES)
    imgs = nc.dram_tensor("imgs", [n_img, P, FW], I32, kind="ExternalInput").ap()
    out = nc.dram_tensor("out", [n_img, P, FW], I32, kind="ExternalOutput").ap()

    iota_u8_d = nc.inline_tensor(
        np.tile(np.arange(256, dtype=np.uint8), (P, 1)), name="iota_u8")
    iota_f32_d = nc.inline_tensor(
        np.tile(np.arange(256, dtype=np.float32), (P, 1)), name="iota_f32")
    # W2_k [(g,b), (g',k')] = [g==g'] * [k'==k] for pass-2 (group collapse
    # with k-slot selection), packed as [P, 16*128] bf16
    import numpy as _np
    w2 = _np.zeros((P, 16, P), _np.float32)
    for k in range(16):
        for g in range(8):
            for b in range(16):
                for kp in range(16):
                    w2[16 * g + b, k, 16 * g + kp] = 1.0 if kp == k else 0.0
    w2_d = nc.inline_tensor(
        _np.ascontiguousarray(w2.reshape(P, 16 * P)).astype(mybir.dt.np(BF16)),
        name="w2")
    acol_d = nc.inline_tensor(
        (_np.arange(P) % 16).astype(_np.float32).reshape(P, 1), name="acol")

    with tile.TileContext(nc) as tc:
        with tc.tile_pool(name="consts", bufs=1) as cpool, \
             tc.tile_pool(name="v8pool", bufs=2) as v8pool, \
             tc.tile_pool(name="hl8pool", bufs=1) as hl8pool, \
             tc.tile_pool(name="v32pool", bufs=1) as v32pool, \
             tc.tile_pool(name="ohpool", bufs=2) as ohpool, \
             tc.tile_pool(name="hpool", bufs=2) as hpool, \
             tc.tile_pool(name="lutpool", bufs=2) as lutpool, \
             tc.tile_pool(name="reppool", bufs=3) as reppool, \
             tc.tile_pool(name="selpool", bufs=3) as selpool, \
             tc.tile_pool(name="lbdpool", bufs=2) as lbdpool, \
             tc.tile_pool(name="psum2", bufs=2, space="PSUM") as psum2, \
             tc.tile_pool(name="psumP1", bufs=2, space="PSUM") as psumP1, \
             tc.tile_pool(name="psumO", bufs=1, space="PSUM") as psumO:

            dma_engines = [nc.sync, nc.scalar, nc.gpsimd]
            iota_u8 = cpool.tile([P, 256], U8)
            nc.sync.dma_start(out=iota_u8[:], in_=iota_u8_d.ap())
            iota_f32 = cpool.tile([P, 256], F32)
            nc.sync.dma_start(out=iota_f32[:], in_=iota_f32_d.ap())
            w2t = cpool.tile([P, 16 * P], BF16)
            nc.sync.dma_start(out=w2t[:], in_=w2_d.ap())
            acol = cpool.tile([P, 1], F32)
            nc.sync.dma_start(out=acol[:], in_=acol_d.ap())

            for _rep in range(repeat):
                for i in range(n_img):
                    # ---------------- Phase A: histogram of image i ----------
                    v32 = v32pool.tile([P, FW], I32, tag="v32")
                    nc.sync.dma_start(out=v32[:], in_=imgs[i, :, :])
                    v8 = v8pool.tile([P, 3, CW], U8, tag="v8")
                    v32v = v32[:].rearrange("p (j c) -> p c j", c=3)
                    for c in range(3):
                        nc.vector.tensor_copy(out=v8[:, c, :], in_=v32v[:, c, :])
                    hl8 = hl8pool.tile([P, 3, 2, CW], U8, tag=f"hl8_{i % 2}")
                    nc.vector.tensor_scalar(out=hl8[:, :, 0, :], in0=v8[:],
                                            scalar1=4, scalar2=None,
                                            op0=AL.logical_shift_right)
                    nc.vector.tensor_scalar(out=hl8[:, :, 1, :], in0=v8[:],
                                            scalar1=15, scalar2=None,
                                            op0=AL.bitwise_and)
                    hi8 = hl8[:, :, 0, :]
                    lo8 = hl8[:, :, 1, :]
                    hps = psum2.tile([48, 48], F32, tag="hps")
                    nmm = CW // j_hist
                    for jc, j0 in enumerate(range(0, CW, j_hist)):
                        ohhi = ohpool.tile([P, j_hist, 3, 16], BF16, tag="ohhi")
                        ohlo = ohpool.tile([P, j_hist, 3, 16], BF16, tag="ohlo")
                        ihi = iota_u8[:, 0:16].unsqueeze(1).unsqueeze(1) \
                            .to_broadcast([P, j_hist, 3, 16])
                        nc.vector.tensor_tensor(
                            out=ohhi[:],
                            in0=hi8[:, :, j0:j0 + j_hist].transpose([0, 2, 1])
                                .unsqueeze(3).to_broadcast([P, j_hist, 3, 16]),
                            in1=ihi, op=AL.is_equal)
                        nc.vector.tensor_tensor(
                            out=ohlo[:],
                            in0=lo8[:, :, j0:j0 + j_hist].transpose([0, 2, 1])
                                .unsqueeze(3).to_broadcast([P, j_hist, 3, 16]),
                            in1=ihi, op=AL.is_equal)
                        for jj in range(j_hist):
                            nc.tensor.matmul(
                                out=hps[:],
                                lhsT=ohhi[:, jj, :, :].rearrange("p c a -> p (c a)"),
                                rhs=ohlo[:, jj, :, :].rearrange("p c a -> p (c a)"),
                                start=(jc == 0 and jj == 0),
                                stop=(jc == nmm - 1 and jj == j_hist - 1))
                    s48 = hpool.tile([48, 48], F32, tag="s48")
                    nc.vector.tensor_copy(out=s48[:], in_=hps[:])
                    hall_i = lutpool.tile([3, 256], F32, tag="hall")
                    for c in range(3):
                        nc.sync.dma_start(
                            out=hall_i[c:c + 1, :],
                            in_=s48[c * 16:(c + 1) * 16, c * 16:(c + 1) * 16])

                    # ---------------- Phase B: LUT construction ----------------
                    lp = lutpool
                    H = lp.tile([3, 256], F32)
                    nc.vector.tensor_copy(out=H[:], in_=hall_i[:])
                    iota_r = iota_f32[0:3, :]

                    ca = lp.tile([3, 256], F32, tag="cs_a")
                    cb = lp.tile([3, 256], F32, tag="cs_b")
                    nc.vector.tensor_copy(out=ca[:], in_=H[:])
                    src, dst = ca, cb
                    sh = 1
                    while sh < 256:
                        nc.vector.tensor_copy(out=dst[:, 0:sh], in_=src[:, 0:sh])
                        nc.vector.tensor_tensor(
                            out=dst[:, sh:256], in0=src[:, sh:256],
                            in1=src[:, 0:256 - sh], op=AL.add)
                        src, dst = dst, src
                        sh *= 2
                    csum = src

                    mask = lp.tile([3, 256], F32, tag="sc1")
                    nc.vector.tensor_scalar(out=mask[:], in0=H[:], scalar1=0.0,
                                            scalar2=None, op0=AL.is_gt)
                    nc.vector.tensor_tensor(out=mask[:], in0=mask[:], in1=iota_r,
                                            op=AL.mult)
                    maxidx = lp.tile([3, 1], F32, tag="sv1")
                    nc.vector.tensor_reduce(out=maxidx[:], in_=mask[:], axis=AX.X,
                                            op=AL.max)
                    nc.vector.tensor_scalar(out=mask[:], in0=iota_r, scalar1=maxidx[:],
                                            scalar2=None, op0=AL.is_equal)
                    nc.vector.tensor_tensor(out=mask[:], in0=mask[:], in1=H[:],
                                            op=AL.mult)
                    lastv = lp.tile([3, 1], F32, tag="sv2")
                    nc.vector.tensor_reduce(out=lastv[:], in_=mask[:], axis=AX.X,
                                            op=AL.add)
                    rem = lp.tile([3, 1], F32, tag="sv3")
                    nc.vector.tensor_scalar(out=rem[:], in0=lastv[:], scalar1=-1.0,
                                            scalar2=float(NPIX), op0=AL.mult,
                                            op1=AL.add)

                    def floordiv_fix(dst_f, x_ap, d_imm, tmp_i, tmp_p, tmp_m):
                        nc.vector.tensor_scalar(out=dst_f[:], in0=x_ap,
                                                scalar1=float(1.0 / d_imm),
                                                scalar2=0.5,
                                                op0=AL.mult, op1=AL.add)
                        nc.vector.tensor_copy(out=tmp_i[:], in_=dst_f[:])
                        nc.vector.tensor_copy(out=dst_f[:], in_=tmp_i[:])
                        for _ in range(2):
                            nc.vector.tensor_scalar(out=tmp_p[:], in0=dst_f[:],
                                                    scalar1=float(d_imm),
                                                    scalar2=None, op0=AL.mult)
                            nc.vector.tensor_tensor(out=tmp_m[:], in0=tmp_p[:],
                                                    in1=x_ap, op=AL.is_gt)
                            nc.vector.tensor_tensor(out=dst_f[:], in0=dst_f[:],
                                                    in1=tmp_m[:], op=AL.subtract)
                        nc.vector.tensor_scalar(out=tmp_p[:], in0=dst_f[:],
                                                scalar1=1.0, scalar2=None,
                                                op0=AL.add)
                        nc.vector.tensor_scalar(out=tmp_p[:], in0=tmp_p[:],
                                                scalar1=float(d_imm), scalar2=None,
                                                op0=AL.mult)
                        nc.vector.tensor_tensor(out=tmp_m[:], in0=tmp_p[:],
                                                in1=x_ap, op=AL.is_le)
                        nc.vector.tensor_tensor(out=dst_f[:], in0=dst_f[:],
                                                in1=tmp_m[:], op=AL.add)

                    s_f = lp.tile([3, 1], F32, tag="sv5")
                    sv_i = lp.tile([3, 1], I32, tag="sv6")
                    sv_p = lp.tile([3, 1], F32, tag="sv6b")
                    sv_m = lp.tile([3, 1], F32, tag="sv6c")
                    floordiv_fix(s_f, rem[:], 255.0, sv_i, sv_p, sv_m)
                    hhalf = lp.tile([3, 1], F32, tag="sv7")
                    floordiv_fix(hhalf, s_f[:], 2.0, sv_i, sv_p, sv_m)
                    s_safe = lp.tile([3, 1], F32, tag="sv8")
                    nc.vector.tensor_scalar(out=s_safe[:], in0=s_f[:], scalar1=1.0,
                                            scalar2=None, op0=AL.max)
                    s_rec = lp.tile([3, 1], F32, tag="sv9")
                    nc.vector.reciprocal(out=s_rec[:], in_=s_safe[:])

                    x = lp.tile([3, 256], F32, tag="sc2")
                    nc.vector.tensor_scalar(out=x[:], in0=csum[:], scalar1=hhalf[:],
                                            scalar2=None, op0=AL.add)
                    q = lp.tile([3, 256], F32, tag="sc3")
                    nc.vector.tensor_scalar(out=q[:], in0=x[:], scalar1=s_rec[:],
                                            scalar2=0.5, op0=AL.mult, op1=AL.add)
                    qi = lp.tile([3, 256], I32, tag="sc4")
                    nc.vector.tensor_copy(out=qi[:], in_=q[:])
                    nc.vector.tensor_copy(out=q[:], in_=qi[:])
                    prod = lp.tile([3, 256], F32, tag="sc5")
                    fm = lp.tile([3, 256], F32, tag="sc6")
                    for _ in range(2):
                        nc.vector.tensor_scalar(out=prod[:], in0=q[:],
                                                scalar1=s_safe[:], scalar2=None,
                                                op0=AL.mult)
                        nc.vector.tensor_tensor(out=fm[:], in0=prod[:], in1=x[:],
                                                op=AL.is_gt)
                        nc.vector.tensor_tensor(out=q[:], in0=q[:], in1=fm[:],
                                                op=AL.subtract)
                    nc.vector.tensor_scalar(out=prod[:], in0=q[:], scalar1=1.0,
                                            scalar2=None, op0=AL.add)
                    nc.vector.tensor_scalar(out=prod[:], in0=prod[:],
                                            scalar1=s_safe[:], scalar2=None,
                                            op0=AL.mult)
                    nc.vector.tensor_tensor(out=fm[:], in0=prod[:], in1=x[:],
                                            op=AL.is_le)
                    nc.vector.tensor_tensor(out=q[:], in0=q[:], in1=fm[:], op=AL.add)
                    nc.vector.tensor_scalar(out=q[:], in0=q[:], scalar1=255.0,
                                            scalar2=0.0, op0=AL.min, op1=AL.max)
                    lutq = lp.tile([3, 256], F32, tag="sc7")
                    nc.vector.memset(lutq[:, 0:1], 0.0)
                    nc.vector.tensor_copy(out=lutq[:, 1:256], in_=q[:, 0:255])
                    szm = lp.tile([3, 1], F32, tag="sv10")
                    nc.vector.tensor_scalar(out=szm[:], in0=s_f[:], scalar1=0.0,
                                            scalar2=None, op0=AL.is_equal)
                    dlt = lp.tile([3, 256], F32, tag="sc8")
                    nc.vector.tensor_tensor(out=dlt[:], in0=iota_r, in1=lutq[:],
                                            op=AL.subtract)
                    nc.vector.scalar_tensor_tensor(
                        out=lutq[:], in0=dlt[:], scalar=szm[:], in1=lutq[:],
                        op0=AL.mult, op1=AL.add)
                    lut16 = lp.tile([3, 256], BF16)
                    nc.vector.tensor_copy(out=lut16[:], in_=lutq[:])


                    # ---------------- Phase C: apply ----------------
                    # Replicated-nibble layout: partition (g, a) holds the data of
                    # source partition 16g+k0 (per k0 round). MM1 gathers the
                    # 16 LUT candidates T[hi, b] via a block-diagonal stationary;
                    # a fused stt selects b == lo; MM2 collapses the b-partitions
                    # and routes the result to partition row 16g+k0 of the PSUM
                    # accumulator, which after 16 rounds is the full output tile.
                    if True:
                        o32 = v32pool.tile([P, FW], I32, tag="v32")
                        o32v = o32[:].rearrange("p (j c) -> p c j", c=3)
                        for c in range(3):
                            r = i * 3 + c
                            # build block-diag stationary LBD[(g,a),(g,b)] = T[a,b]
                            lbd = lbdpool.tile([P, P], BF16, tag="lbd")
                            nc.vector.memset(lbd[:], 0.0)
                            for g in range(8):
                                nc.sync.dma_start(
                                    out=lbd[16 * g:16 * (g + 1), 16 * g:16 * (g + 1)],
                                    in_=lut16[c:c + 1, :])
                            out2 = psumO.tile([P, CW], F32, tag="out2")
                            for k0 in range(16):
                                hlrep = reppool.tile([P, 2, CW], U8, tag="hlrep")
                                for g in range(8):
                                    src = hl8[16 * g + k0:16 * g + k0 + 1,
                                                  c, :, :]
                                    eng = dma_engines[(k0 * 8 + g) % len(dma_engines)]
                                    eng.dma_start(
                                        out=hlrep[16 * g:16 * (g + 1), :, :],
                                        in_=src.rearrange("p a b -> p (a b)")
                                            .unsqueeze(1)
                                            .to_broadcast([1, 16, 2 * CW]))
                                hirep = hlrep[:, 0, :]
                                lorep = hlrep[:, 1, :]
                                ohhit = selpool.tile([P, CW], BF16, tag="ohhit")
                                nc.vector.tensor_scalar(
                                    out=ohhit[:], in0=hirep[:], scalar1=acol[:],
                                    scalar2=None, op0=AL.is_equal)
                                for q4, q0 in enumerate(range(0, CW, 512)):
                                    p1 = psumP1.tile([P, 512], F32, tag="p1")
                                    nc.tensor.matmul(
                                        out=p1[:], lhsT=lbd[:],
                                        rhs=ohhit[:, q0:q0 + 512],
                                        start=True, stop=True)
                                    sel = selpool.tile([P, 512], BF16, tag="sel")
                                    nc.vector.scalar_tensor_tensor(
                                        out=sel[:], in0=lorep[:, q0:q0 + 512],
                                        scalar=acol[:], in1=p1[:],
                                        op0=AL.is_equal, op1=AL.mult)
                                    nc.tensor.matmul(
                                        out=out2[:, q0:q0 + 512],
                                        lhsT=w2t[:, k0 * P:(k0 + 1) * P],
                                        rhs=sel[:],
                                        start=(k0 == 0), stop=(k0 == 15))
                            nc.vector.tensor_copy(out=o32v[:, c, :], in_=out2[:])
                        nc.scalar.dma_start(out=out[i, :, :], in_=o32[:])

    nc.compile()
    return nc


_NC_CACHE = {}


def kernel(images: np.ndarray) -> np.ndarray:
    """images: [64, 512, 512, 3] int32 in [0, 255]. Returns equalized images."""
    images = np.asarray(images)
    assert images.shape == (64, 512, 512, 3), images.shape
    assert images.dtype == np.int32
    if "nc" not in _NC_CACHE:
        _NC_CACHE["nc"] = _build_kernel()
    nc = _NC_CACHE["nc"]

    in_maps = [
        {"imgs": np.ascontiguousarray(
            images[c * N_IMG:(c + 1) * N_IMG].reshape(N_IMG, P, FW))}
        for c in range(N_CORES)
    ]
    res = run_bass_kernel_spmd(nc, in_maps, list(range(N_CORES)))
    out = np.concatenate(
        [res.results[c]["out"].reshape(N_IMG, 512, 512, 3)
         for c in range(N_CORES)], axis=0)
    return out.astype(np.int32)

